# revision 21
# baseline (speedup 1.0000x reference)
import sys

if "/opt/trn_rl_repo" not in sys.path:
    sys.path.insert(0, "/opt/trn_rl_repo")

import numpy as np

T = 8192
D = 1024
NCORES = 8
RPC = T // NCORES          # output rows per core = 1024
OUTR = 121                 # output rows per tile (128 input partitions -> 121 stencil rows)
TILES = 9                  # ceil(1024 / 121)
HALO = 3                   # left halo: out row i needs input rows i-3 .. i+4
SHARD = OUTR * (TILES - 1) + 128   # 1096 input rows per core (zero-padded)
NST = 8                    # stencils: k0,k1,k2, cA0..cA3, coarse_dwt
INV_SQRT2 = np.float32(0.7071067811865476)


def _fft_C():
    # Replicates reference: C = cos(2*pi*n*k/4) computed in fp32
    n4 = np.arange(4)
    ang = (
        np.float32(2.0 * np.pi)
        * (n4[:, None] * n4[None, :]).astype(np.float32)
        / np.float32(4.0)
    ).astype(np.float32)
    return np.cos(ang).astype(np.float32)  # [k, n]


def build_stencil():
    """[128, NST*OUTR] fp32. Column block m, col r: coefficients over input
    partitions k such that  out_m[r] = sum_k st[k, m*OUTR+r] * y[k].

    Local geometry: out row r (global i) <-> y[r+n] = x[i-3+n], n=0..7.
    fft window w_n = x[i-1+n] = y[r+2+n], n=0..3.
    """
    st = np.zeros((128, NST * OUTR), np.float32)
    r = np.arange(OUTR)
    C = _fft_C()
    # m=0..2: fft k-blocks (k3 == k1 up to ~1e-8 coefficient dust; copied)
    for k in range(3):
        for n in range(4):
            st[r + 2 + n, k * OUTR + r] = C[k, n]
    # m=3..6: cA_j = (y[r+2j] + y[r+2j+1]) / sqrt(2)
    for j in range(4):
        st[r + 2 * j, (3 + j) * OUTR + r] = INV_SQRT2
        st[r + 2 * j + 1, (3 + j) * OUTR + r] = INV_SQRT2
    # m=7: coarse_dwt = sum_{n<8} y[r+n] / (4*sqrt(2))
    for n in range(8):
        st[r + n, 7 * OUTR + r] = INV_SQRT2 / np.float32(4.0)
    return st


_COMPUTE_OPS = ("Matmult", "Activation", "TensorCopy", "Memset", "ISA")
# Empirical walrus descriptor capacities (setupSyncWait fails above these):
# fused fp32 Matmult (S3_LW), DMACopy (DMA_DIRECT2D) and TensorCopy
# (S4D4_TR) each hold ONE embedded sem wait; assume the same for
# Activation rather than risk another compile failure.
_WAIT_CAPS = {"Matmult": 1, "DMACopy": 1, "Activation": 1, "TensorCopy": 1}
_N_NOPS = 13  # Pool-engine nops that absorb the final drain's sem waits


def _fix_sync_waits(nc, verbose=False):
    """Make every instruction fit its descriptor's sync-wait capacity.

    Ordering model (matches the tile framework's own elisions and the
    CoreSim race detector):
      - each compute engine dispatches its queue in order and blocks at an
        un-passed wait, so waits on earlier queue-mates carry forward;
        instructions complete in order within an engine;
      - ALL HW-DGE DMAs here are issued on the single SP ring, which
        evaluates embedded waits in descriptor order (the framework already
        relies on this: some DMAs carry no wait because a ring predecessor
        waited). Treated as one "RING" queue for wait-carry;
      - a DMAHW sem's value counts completed DMAs on that sem IN ORDER
        because every sem reuse carries a chain wait (sem >= previous
        value) gating its launch on the predecessor's completion. So both
        engine sems and DMA sems give prefix knowledge: waiting (s >= v)
        implies every event up to cumulative v completed, with everything
        those events' instructions had themselves waited on.

    Phase A (strip): fixpoint-drop any wait implied by the instruction's
    other kept waits + waits carried from earlier queue positions.

    Phase B (redistribute): move an excess wait W off over-cap X either
      (1) onto an earlier same-queue instruction with spare capacity
          (queue blocks there instead -- strictly earlier, still before X), or
      (2) onto a compute instruction H whose completion the queue already
          awaits at-or-before X: some kept wait (E >= v) on X or a queue
          predecessor implies E-queue position p(v) completed, so any H at
          position <= p(v) with spare capacity completes first; hosting W
          on H means W has passed before the queue passes that wait. The
          derivation wait is pinned to its queue (may still move earlier
          within it, never off it).
    Both directions require every update satisfying W to be scheduled
    strictly before the host (so the wait can pass; no deadlock), which
    also preserves every transitive elision made by the framework or
    Phase A: W's condition still holds at X's queue position.
    """
    insts = list(nc.all_instructions())

    def queue_of(i):
        if i.opcode == "DMACopy":
            return "RING"
        if i.opcode in _COMPUTE_OPS:
            return str(i.engine)
        return None

    updater_queues = {}
    updaters = {}  # sem -> [(inst_idx, add)] in program order
    for n, i in enumerate(insts):
        si = i.sync_info
        if si is None:
            continue
        q = queue_of(i)
        for u in si.on_update:
            if u.update_mode == "sem-inc":
                add = 1
            elif u.update_mode == "sem-add-imm":
                add = u.update_value
            else:
                updater_queues.setdefault(u.id, set()).add("?poison")
                continue
            updater_queues.setdefault(u.id, set()).add(q if q else "?other")
            updaters.setdefault(u.id, []).append((n, add))

    def sem_queue(s):
        e = updater_queues.get(s, set())
        if len(e) == 1:
            (q,) = e
            if q not in ("?poison", "?other"):
                return q
        return None

    def merge(dst, src):
        for s, v in src.items():
            if dst.get(s, 0) < v:
                dst[s] = v

    def plain(w):
        return w.sync_type == "semaphore" and w.wait_mode == "sem-ge-imm"

    events = {}  # sem -> [(cum_after, knowledge_dict)]
    queue_know = {}
    queue_contrib = {}

    def wait_knowledge(s, v):
        k = {s: v}
        if sem_queue(s) is None:
            return k
        for cum, ek in events.get(s, []):
            merge(k, ek)
            k[s] = max(k[s], cum)
            if cum >= v:
                break
        return k

    # ---- Phase A: fixpoint strip ----
    for i in insts:
        si = i.sync_info
        if si is None:
            continue
        q = queue_of(i)
        base = dict(queue_know.get(q, {})) if q else {}
        waits = list(si.on_wait)
        if i.opcode in _WAIT_CAPS:
            changed = True
            while changed:
                changed = False
                for w in waits:
                    if not plain(w) or sem_queue(w.id) is None:
                        continue
                    K = dict(base)
                    for w2 in waits:
                        if w2 is not w and plain(w2):
                            merge(K, wait_knowledge(w2.id, w2.wait_value))
                    if K.get(w.id, 0) >= w.wait_value:
                        waits.remove(w)
                        changed = True
                        break
        if len(waits) != len(si.on_wait):
            si.on_wait = waits
        know = base
        for w in waits:
            if plain(w):
                merge(know, wait_knowledge(w.id, w.wait_value))
        if q:
            queue_know[q] = dict(know)
        for u in si.on_update:
            if u.update_mode == "sem-inc":
                add = 1
            elif u.update_mode == "sem-add-imm":
                add = u.update_value
            else:
                continue
            ek = dict(know)
            if q and q != "RING":
                # in-order completion within a compute engine: this event
                # implies the engine's earlier updates to every sem
                ec = queue_contrib.setdefault(q, {})
                ec[u.id] = ec.get(u.id, 0) + add
                merge(ek, ec)
            prev = events[u.id][-1][0] if events.get(u.id) else 0
            events.setdefault(u.id, []).append((prev + add, ek))

    # ---- Phase B: redistribute excess waits ----
    queues = {}  # queue name -> [inst_idx] in order
    qpos = {}
    for n, i in enumerate(insts):
        q = queue_of(i)
        if q and i.sync_info is not None:
            lst = queues.setdefault(q, [])
            qpos[n] = len(lst)
            lst.append(n)

    cur = {n: list(insts[n].sync_info.on_wait) for n in qpos}
    pinned = set()  # id(wait): load-bearing for some derivation; don't move

    def cap_of(n):
        return _WAIT_CAPS.get(insts[n].opcode, 2)

    def spare(n):
        return cap_of(n) - len(cur[n])

    def satisfier_bound(w):
        cum = 0
        for n, add in updaters.get(w.id, []):
            cum += add
            if cum >= w.wait_value:
                return n
        return len(insts)

    def pos_of_value(s, v):
        """Queue position (index into queues[E]) of the instruction whose
        completion brings sem s to >= v, or None."""
        cum = 0
        for n, add in updaters.get(s, []):
            cum += add
            if cum >= v:
                return qpos.get(n)
        return None

    def find_host(x, W, seed_waits):
        """A host H may absorb wait W from X iff W is guaranteed passed
        before X proceeds: H sits at-or-before a queue position whose
        dispatch/completion X already (transitively) awaits. Windows are
        grown BFS: a kept wait (s >= v) on any windowed instruction opens
        the window of s's queue up to the satisfier position (queue members
        there dispatch -- waits passed -- before that satisfier completes).
        All waits on the derivation path get pinned by the caller."""
        bw = satisfier_bound(W)
        best = {}  # queue -> (pmax, via-chain of wait objs)
        work = []

        def add(qn, pmax, via):
            if qn is None or pmax is None:
                return
            old = best.get(qn)
            if old is not None and old[0] >= pmax:
                return
            best[qn] = (pmax, via)
            work.append((qn, pmax, via))

        if qpos[x] > 0:
            add(queue_of(insts[x]), qpos[x] - 1, ())
        for w2 in seed_waits:
            if plain(w2):
                add(sem_queue(w2.id), pos_of_value(w2.id, w2.wait_value), (w2,))
        while work:
            qn, pmax, via = work.pop()
            if best.get(qn, (None,))[0] != pmax:
                continue  # superseded by a larger window
            ql2 = queues[qn]
            for hp in range(min(pmax, len(ql2) - 1), -1, -1):
                hn = ql2[hp]
                if hn == x:
                    continue
                for w2 in cur[hn]:
                    if w2 is W or not plain(w2):
                        continue
                    add(
                        sem_queue(w2.id),
                        pos_of_value(w2.id, w2.wait_value),
                        via + (w2,),
                    )
        for qn, (pmax, via) in best.items():
            ql2 = queues[qn]
            for hp in range(min(pmax, len(ql2) - 1), -1, -1):
                hn = ql2[hp]
                if hn <= bw:
                    break  # queue list ascends in program order
                if hn == x:
                    continue
                if spare(hn) > 0:
                    return hn, via
        return None, None

    moved, failed = 0, []
    for x in sorted(qpos, key=lambda n: n):
        cap = _WAIT_CAPS.get(insts[x].opcode)
        if cap is None:
            continue
        while len(cur[x]) > cap:
            movable = [
                w
                for w in cur[x]
                if plain(w) and id(w) not in pinned and sem_queue(w.id)
            ]
            placed = False
            for W in sorted(movable, key=satisfier_bound):
                seed = [w for w in cur[x] if w is not W and plain(w)]
                hn, via = find_host(x, W, seed)
                if hn is not None:
                    cur[hn].append(W)
                    cur[x].remove(W)
                    for wv in via:
                        pinned.add(id(wv))
                    placed = True
                    break
            if placed:
                moved += 1
            else:
                failed.append((insts[x].name, insts[x].opcode, len(cur[x])))
                break

    for n in qpos:
        if len(cur[n]) != len(insts[n].sync_info.on_wait):
            insts[n].sync_info.on_wait = cur[n]

    # ---- Drain surgery: the final SP drain's descriptor holds one wait,
    # but the framework gives it one per sem final (11+). Park each
    # DMAHW/PE/DVE/ACT final on a dedicated Pool-engine nop: nothing in
    # the program depends on Pool progress, so this cannot deadlock, and
    # Pool completes in order, so the drain's single kept wait
    # (Pool >= final) implies every absorbed wait held.
    big = [
        i
        for i in insts
        if i.opcode == "Drain"
        and i.sync_info is not None
        and any("DMAHW" in w.ant_name for w in i.sync_info.on_wait if plain(w))
    ]
    if big:
        (dr,) = big
        dwaits = list(dr.sync_info.on_wait)
        keep = [w for w in dwaits if not plain(w) or w.ant_name.startswith("Pool")]
        move_w = [w for w in dwaits if plain(w) and not w.ant_name.startswith("Pool")]
        assert sum(1 for w in keep if plain(w)) == 1, [
            (w.ant_name, w.wait_value) for w in dwaits
        ]
        nops = [
            i
            for i in insts
            if i.opcode == "Memset"
            and "Pool" in str(i.engine)
            and i.sync_info is not None
            and not i.sync_info.on_wait
            and any(u.ant_name.startswith("Pool") for u in i.sync_info.on_update)
        ]
        assert len(nops) >= len(move_w), (len(nops), len(move_w))
        for nop, w in zip(nops, move_w):
            nop.sync_info.on_wait = [w]
        dr.sync_info.on_wait = keep

    over = [
        (i.name, i.opcode, len(i.sync_info.on_wait))
        for i in insts
        if i.opcode in _WAIT_CAPS
        and i.sync_info is not None
        and len(i.sync_info.on_wait) > _WAIT_CAPS[i.opcode]
    ]
    if verbose or over:
        print(f"_fix_sync_waits: moved={moved} over_cap={over} failed={failed}")
    if over:
        raise RuntimeError(f"sync waits over descriptor capacity: {over}")


def _build_nc():
    import concourse.bass as bass
    import concourse.tile as tile
    from concourse import mybir

    f32 = mybir.dt.float32
    nc = bass.Bass()
    xs = nc.dram_tensor("xs", [SHARD, D], f32, kind="ExternalInput")
    st = nc.dram_tensor("st", [128, NST * OUTR], f32, kind="ExternalInput")
    cf = nc.dram_tensor("cf", [RPC, D], f32, kind="ExternalOutput")
    ff = nc.dram_tensor("ff", [RPC, 4 * D], f32, kind="ExternalOutput")
    cd = nc.dram_tensor("cd", [RPC, D], f32, kind="ExternalOutput")
    fd = nc.dram_tensor("fd", [RPC, 4 * D], f32, kind="ExternalOutput")

    Copy = mybir.ActivationFunctionType.Copy

    # Wait-capacity discipline (fused fp32 Matmult holds ONE embedded wait,
    # DMACopy holds two): each staging tile has a single producer engine and
    # a single reader, so every copy/DMA needs at most {producer, WAR} = 2
    # waits; a 1x1 dummy matmul absorbs the y-DMA wait for the tile's real
    # matmuls; _fix_sync_waits handles the leftovers (PSUM WAW on matmuls).
    # fft staging (cols k0,k1,k2) is written by ACT only; fdw/cdw by DVE
    # only; the k3(==k1) block gets its own ACT copy of pair-1's PSUM into
    # fk3 and its own output DMA.
    with tile.TileContext(nc) as tc:
        with (
            tc.tile_pool(name="const", bufs=1) as cpool,
            tc.tile_pool(name="io", bufs=2) as io,
            tc.tile_pool(name="psum", bufs=3, space="PSUM") as pp,
            tc.tile_pool(name="scr", bufs=1, space="PSUM") as sp,
        ):
            S = cpool.tile([128, NST * OUTR], f32)
            nc.sync.dma_start(S[:], st[:])
            for t in range(TILES):
                nrows = OUTR if t < TILES - 1 else RPC - (TILES - 1) * OUTR
                y = io.tile([128, D], f32, tag="y")
                nc.sync.dma_start(y[:], xs[OUTR * t : OUTR * t + 128, :])
                # coarse_fft[i] = x[i-1] = y[r+2]: straight copy out of SBUF
                nc.sync.dma_start(cf[OUTR * t : OUTR * t + nrows, :], y[2 : 2 + nrows, :])

                scr = sp.tile([1, 8], f32, tag="z")
                nc.tensor.matmul(scr[0:1, 0:1], y[0:1, 0:1], y[0:1, 0:1], start=True, stop=True)

                fft = io.tile([OUTR, 3 * D], f32, tag="ff")
                fk3 = io.tile([OUTR, D], f32, tag="f3")
                fdw = io.tile([OUTR, 4 * D], f32, tag="fd")
                cdw = io.tile([OUTR, D], f32, tag="cd")
                dests = [
                    (fft, 0), (fft, D), (fft, 2 * D),
                    (fdw, 0), (fdw, D), (fdw, 2 * D), (fdw, 3 * D),
                    (cdw, 0),
                ]
                for m in range(NST):
                    P = pp.tile([OUTR, D], f32, tag="p")
                    lhsT = S[:, m * OUTR : (m + 1) * OUTR]
                    nc.tensor.matmul(P[:, 0:512], lhsT, y[:, 0:512], start=True, stop=True)
                    nc.tensor.matmul(P[:, 512:D], lhsT, y[:, 512:D], start=True, stop=True)
                    dst, col = dests[m]
                    if m < 3:
                        nc.scalar.activation(dst[:, col : col + D], P[:], Copy)
                    else:
                        nc.vector.tensor_copy(dst[:, col : col + D], P[:])
                    if m == 1:
                        nc.scalar.activation(fk3[:], P[:], Copy)

                nc.sync.dma_start(ff[OUTR * t : OUTR * t + nrows, 0 : 3 * D], fft[0:nrows, :])
                nc.sync.dma_start(ff[OUTR * t : OUTR * t + nrows, 3 * D : 4 * D], fk3[0:nrows, :])
                nc.sync.dma_start(cd[OUTR * t : OUTR * t + nrows, :], cdw[0:nrows, :])
                nc.sync.dma_start(fd[OUTR * t : OUTR * t + nrows, :], fdw[0:nrows, :])
            # Drain-wait fanout: the final SP drain's descriptor holds ONE
            # wait, but must await 8 DMAHW finals + PE + DVE + ACT. These
            # Pool-engine nops (no inputs, disjoint outputs, zero deps)
            # exist solely so _fix_sync_waits can park one final-wait on
            # each; nothing depends on Pool progress, so this can never
            # deadlock, and the drain's single kept wait (Pool >= final)
            # implies all of them held (Pool completes in order).
            scratch = cpool.tile([1, _N_NOPS + 2], f32)
            for j in range(_N_NOPS):
                nc.gpsimd.memzero(scratch[0:1, j : j + 1])
    nc.finalize()
    _fix_sync_waits(nc)
    return nc


def make_shards(x2):
    shards = []
    for c in range(NCORES):
        lo = RPC * c - HALO
        sh = np.zeros((SHARD, D), np.float32)
        a, b = max(lo, 0), min(lo + SHARD, T)
        sh[a - lo : b - lo] = x2[a:b]
        shards.append(sh)
    return shards


def fix_boundary(x2, cf, ff, cd, fd):
    """Left-edge rows where the reference anchors (clamps) the window start:
    fft row 0, dwt rows 0..2. Computed exactly as the reference does."""
    C = _fft_C()
    w = np.zeros((4, D), np.float32)
    w[0:3] = x2[0:3]
    re = (C @ w).astype(np.float32)  # [k, D]
    cf[0] = re.mean(axis=0, dtype=np.float32)
    ff[0] = re.reshape(-1)
    for i in range(3):
        w8 = x2[0:8].copy()
        w8[np.arange(8) > i + 4] = 0
        cA = ((w8[0::2] + w8[1::2]) * INV_SQRT2).astype(np.float32)  # [4, D]
        cd[i] = cA.mean(axis=0, dtype=np.float32)
        fd[i] = cA.reshape(-1)


_CACHE = {}


def run(x, trace=False):
    from concourse.bass_utils import run_bass_kernel_spmd

    x = np.asarray(x)
    assert x.shape == (1, T, D), x.shape
    x2 = np.ascontiguousarray(x.reshape(T, D)).astype(np.float32, copy=False)

    if "nc" not in _CACHE:
        _CACHE["nc"] = _build_nc()
    nc = _CACHE["nc"]

    stn = build_stencil()
    in_maps = [{"xs": sh, "st": stn} for sh in make_shards(x2)]
    res = run_bass_kernel_spmd(nc, in_maps, list(range(NCORES)), trace=trace)
    rs = res.results
    cf = np.concatenate([rs[c]["cf"] for c in range(NCORES)], axis=0)
    ff = np.concatenate([rs[c]["ff"] for c in range(NCORES)], axis=0)
    cd = np.concatenate([rs[c]["cd"] for c in range(NCORES)], axis=0)
    fd = np.concatenate([rs[c]["fd"] for c in range(NCORES)], axis=0)
    fix_boundary(x2, cf, ff, cd, fd)
    out = (
        cf.reshape(1, T, D),
        ff.reshape(1, T, 4 * D),
        cd.reshape(1, T, D),
        fd.reshape(1, T, 4 * D),
    )
    return out, res


def kernel(x):
    out, _ = run(x, trace=False)
    return out
